# revision 1
# baseline (speedup 1.0000x reference)
"""Trainium2 Bass kernel for nn_MultiHeadAttn (dense transformer block:
QKV proj -> causal MHA -> out proj -> residual -> LayerNorm).

Sharding: tensor-parallel over the 16 heads across 8 NeuronCores (2 heads
per core). Each core computes Q/K/V projections for its heads over all
tokens, flash-style causal attention (scores kept transposed [k, q] so the
softmax denominator comes from an appended ones-column in V), then the
normalized per-head attention vectors are exchanged with an on-chip
AllToAll so that each core holds all 16 heads for 1/8 of the token rows.
Each core then applies the output projection, residual add and LayerNorm
for its token rows. The host only slices/stacks inputs and concatenates
the 8 output chunks.
"""

import os
import sys

import numpy as np

try:
    import concourse.bass as bass  # noqa: F401
except ImportError:  # pragma: no cover
    sys.path.insert(0, "/opt/trn_rl_repo")

import ml_dtypes

import concourse.bass as bass
import concourse.mybir as mybir
import concourse.tile as tile
from concourse import bacc
from concourse.bass_utils import run_bass_kernel_spmd
from concourse.masks import make_upper_triangular

# Problem constants
T_FULL = 2048
B = 2
D_MODEL = 1024
N_HEAD = 16
D_HEAD = 64
LN_EPS = 1e-5
N_CORES = 8
SCALE = 1.0 / (D_HEAD**0.5)
EXP_BIAS = -3.0  # scores are in [-3.3, 3.3] for this problem; keeps exp <= ~1.4

P = 128
KCH = D_MODEL // P  # 8 contraction chunks
IB = 512  # i-block (query block) width

F32 = mybir.dt.float32
BF16 = mybir.dt.bfloat16

# Stash of the most recent run's BassKernelResults (for test harnesses).
LAST_RESULT = None


def build_program(t=T_FULL, n_cores=N_CORES, repeat=1, no_collective=False, apply_gb=True):
    """Builds the SPMD Bass program (same program on every core).

    repeat > 1 re-emits the whole kernel body (everything except constant
    weight loads) that many times — used only for wall-clock timing.
    """
    nh_loc = 2  # heads per core
    n_ib = t // IB  # i-blocks per batch
    nt = t // P  # 128-token tiles per batch
    cs = t // n_cores  # per-batch token chunk per core (A2A shard)
    assert cs % P == 0, "need t >= 128*n_cores for per-batch A2A tiling"
    tiles_pb = cs // P  # 128-row output tiles per batch per core
    n_it = B * tiles_pb  # 128-row output tiles per core

    nc = bacc.Bacc(
        "TRN2", target_bir_lowering=False, debug=False, num_devices=n_cores
    )

    # Kernel I/O (per-core tensors; host supplies per-core contents)
    hT_d = nc.dram_tensor("hT", [B, KCH, P, t], BF16, kind="ExternalInput").ap()
    wqk_d = nc.dram_tensor("wqk", [KCH, P, 2 * nh_loc * D_HEAD], BF16, kind="ExternalInput").ap()
    wv_d = nc.dram_tensor("wv", [KCH, P, nh_loc * D_HEAD], BF16, kind="ExternalInput").ap()
    wo_d = nc.dram_tensor("wo", [KCH, P, D_MODEL], BF16, kind="ExternalInput").ap()
    hres_d = nc.dram_tensor("hres", [n_it, P, D_MODEL], F32, kind="ExternalInput").ap()
    g_d = nc.dram_tensor("lng", [D_MODEL], F32, kind="ExternalInput").ap()
    b_d = nc.dram_tensor("lnb", [D_MODEL], F32, kind="ExternalInput").ap()
    out_d = nc.dram_tensor("out", [n_it, P, D_MODEL], F32, kind="ExternalOutput").ap()

    with tile.TileContext(nc) as tc:
        with (
            tc.tile_pool(name="consts", bufs=1) as consts,
            tc.tile_pool(name="hpool", bufs=1) as hpool,
            tc.tile_pool(name="qkvp", bufs=1) as qkvp,
            tc.tile_pool(name="expp", bufs=6) as expp,
            tc.tile_pool(name="work", bufs=3) as work,
            tc.tile_pool(name="defer", bufs=n_it) as defer_pool,
            tc.tile_pool(name="avsb", bufs=3) as avsb,
            tc.tile_pool(name="pproj", bufs=2, space="PSUM") as pproj,
            tc.tile_pool(name="psc", bufs=2, space="PSUM") as psc,
            tc.tile_pool(name="pav", bufs=2, space="PSUM") as pav,
            tc.tile_pool(name="dram", bufs=1, space="DRAM") as dram,
        ):
            # ---- constants / weights needed for the first phases ----
            # (wo / g / b / hres are only needed after the AllToAll; their
            # DMAs are emitted late so they don't delay the hT load.)
            wqk_sb = consts.tile([P, KCH, 2 * nh_loc * D_HEAD], BF16)
            wv_sb = consts.tile([P, KCH, nh_loc * D_HEAD], BF16)
            for k in range(KCH):
                nc.sync.dma_start(out=wqk_sb[:, k, :], in_=wqk_d[k])
                nc.sync.dma_start(out=wv_sb[:, k, :], in_=wv_d[k])

            eps_sb = consts.tile([P, 1], F32)
            nc.vector.memset(eps_sb, LN_EPS)
            expb_sb = consts.tile([P, 1], F32)
            nc.vector.memset(expb_sb, EXP_BIAS)

            # [128,128] bf16 mask: 1.0 where j <= i (upper triangle incl diag)
            m1 = consts.tile([P, P], BF16)
            make_upper_triangular(nc, m1, val=1.0, diag=True)
            # [128,256] mask for the odd diagonal tile of a pair:
            # cols 0:128 all-zero (fully masked), cols 128:256 triangle
            m2 = consts.tile([P, 2 * P], BF16)
            nc.gpsimd.memset(m2[:, 0:P], 0.0)
            make_upper_triangular(nc, m2[:, P : 2 * P], val=1.0, diag=True)

            wo_sb = consts.tile([P, KCH, D_MODEL], BF16)
            hres_sb = consts.tile([P, n_it, D_MODEL], F32)
            if apply_gb:
                g_sb = consts.tile([P, D_MODEL], F32)
                b_sb = consts.tile([P, D_MODEL], F32)

            for _rep in range(repeat):
                # ---- hT load ----
                hT_sb = hpool.tile([P, B * KCH, t], BF16)
                for b in range(B):
                    for nb4 in range(t // 512):
                        for k in range(KCH):
                            nc.sync.dma_start(
                                out=hT_sb[:, b * KCH + k, nb4 * 512 : (nb4 + 1) * 512],
                                in_=hT_d[b, k, :, nb4 * 512 : (nb4 + 1) * 512],
                            )

                # ---- Q/K/V projections, batch-major so batch-0 attention
                # can start while batch 1 is still projecting ----
                qT_sb = qkvp.tile([P, B, t], BF16)
                kT_sb = qkvp.tile([P, B, t], BF16)
                # vext[b][h]: [128, nt, 65]; col 64 stays 1.0 (sumexp trick)
                vext = [[None, None] for _ in range(B)]
                for b in range(B):
                    for h in range(nh_loc):
                        v = qkvp.tile([P, nt, D_HEAD + 1], BF16, name=f"vext_{b}_{h}")
                        nc.vector.memset(v[:, :, D_HEAD : D_HEAD + 1], 1.0)
                        vext[b][h] = v
                # interleave q/k/v by 512-token group so attention on the
                # first i-block can begin as soon as group 0 is projected
                for b in range(B):
                    for nb in range(t // 512):
                        for mt in range(2):  # 0 -> q, 1 -> k
                            ps = pproj.tile([P, 512], F32, tag="proj", name="ps_qk")
                            for k in range(KCH):
                                nc.tensor.matmul(
                                    ps,
                                    lhsT=wqk_sb[:, k, mt * P : (mt + 1) * P],
                                    rhs=hT_sb[:, b * KCH + k, nb * 512 : (nb + 1) * 512],
                                    start=(k == 0),
                                    stop=(k == KCH - 1),
                                )
                            dst = (qT_sb if mt == 0 else kT_sb)[:, b, nb * 512 : (nb + 1) * 512]
                            nc.vector.tensor_copy(dst, ps)
                        for tt in range(4 * nb, 4 * nb + 4):
                            ps = pproj.tile([P, 512], F32, tag="proj", name="ps_v")
                            psv = ps[:, : nh_loc * D_HEAD]
                            for k in range(KCH):
                                nc.tensor.matmul(
                                    psv,
                                    lhsT=hT_sb[:, b * KCH + k, tt * P : (tt + 1) * P],
                                    rhs=wv_sb[:, k, :],
                                    start=(k == 0),
                                    stop=(k == KCH - 1),
                                )
                            for h in range(nh_loc):
                                nc.vector.tensor_copy(
                                    vext[b][h][:, tt, 0:D_HEAD],
                                    psv[:, h * D_HEAD : (h + 1) * D_HEAD],
                                )

                # ---- A2A buffers (one exchange per batch) ----
                av_in = [
                    dram.tile([n_cores, P, cs], BF16, name=f"av_in{b}") for b in range(B)
                ]
                av_out = [
                    dram.tile([n_cores, P, cs], BF16, name=f"av_out{b}") for b in range(B)
                ]

                nsub = D_MODEL // 512  # bn_stats subgroups
                po_parity = 0
                deferred = []

                # ---- attention ----
                # scores kept transposed: s[j, i] for j-tile (128 keys) x i-block
                # (512 queries); softmax over j via ones-column in V.
                for b in range(B):
                    for ib in range(n_ib):
                        njt = 4 * ib + 4  # causal: j-tiles 0..4ib+3
                        avps = [
                            pav.tile([D_HEAD + 1, 512], F32, tag="av", name=f"avps{h}")
                            for h in range(nh_loc)
                        ]
                        njp = njt // 2

                        def pair_off(jp):
                            # causal trim: both tiles of a pair compute query
                            # columns [o0, 512); the odd tile's extra 128
                            # columns are invalid and masked via m2.
                            jt0, jt1 = 2 * jp, 2 * jp + 1
                            o0 = max(0, jt0 * P - ib * IB)
                            o1 = max(0, jt1 * P - ib * IB)
                            return jt0, jt1, o0, o1, jt1 * P - ib * IB >= 0

                        def emit_scores(jp, h):
                            jt0, jt1, o0, _, _ = pair_off(jp)
                            base = h * D_HEAD
                            scp = psc.tile([P, 2, 512], F32, tag="sc", name="scp")
                            for jj, jt in ((0, jt0), (1, jt1)):
                                nc.tensor.matmul(
                                    scp[:, jj, o0:512],
                                    lhsT=kT_sb[base : base + D_HEAD, b, jt * P : (jt + 1) * P],
                                    rhs=qT_sb[base : base + D_HEAD, b, ib * IB + o0 : (ib + 1) * IB],
                                    start=True,
                                    stop=True,
                                )
                            return scp

                        # software pipeline: the next pair's score matmuls are
                        # emitted BEFORE this pair's AV matmuls so the PE feeds
                        # the (pacing) Scalar engine as early as possible
                        scp_cur = [emit_scores(0, h) for h in range(nh_loc)]
                        for jp in range(njp):
                            jt0, jt1, o0, o1, diag = pair_off(jp)
                            expts = []
                            for h in range(nh_loc):
                                expt = expp.tile([P, 2, 512], BF16, tag="exp", name="expt")
                                nc.scalar.activation(
                                    expt[:, :, o0:512],
                                    scp_cur[h][:, :, o0:512],
                                    mybir.ActivationFunctionType.Exp,
                                    bias=expb_sb,
                                )
                                expts.append(expt)
                            if jp + 1 < njp:
                                scp_cur = [emit_scores(jp + 1, h) for h in range(nh_loc)]
                            for h in range(nh_loc):
                                expt = expts[h]
                                if diag:
                                    nc.vector.tensor_mul(
                                        expt[:, 0, o0 : o0 + P], expt[:, 0, o0 : o0 + P], m1
                                    )
                                    nc.vector.tensor_mul(
                                        expt[:, 1, o0 : o0 + 2 * P],
                                        expt[:, 1, o0 : o0 + 2 * P],
                                        m2,
                                    )
                                for jj, jt, oj in ((0, jt0, o0), (1, jt1, o1)):
                                    nc.tensor.matmul(
                                        avps[h][:, oj:512],
                                        lhsT=vext[b][h][:, jt, :],
                                        rhs=expt[:, jj, oj:512],
                                        start=(jt == 0),
                                        stop=(jt == njt - 1),
                                    )
                        # normalize by sumexp (row 64) and ship to the A2A buffer
                        for h in range(nh_loc):
                            # sumexp row: PSUM@p64 -> SBUF@p0 copy (exact),
                            # then reciprocal from SBUF@p0 (approx_fast can't
                            # read shifted PSUM), then broadcast from p0.
                            srow = work.tile([1, 512], F32, tag="srow", name="srow")
                            nc.vector.tensor_copy(srow, avps[h][D_HEAD : D_HEAD + 1, :])
                            rt = work.tile([1, 512], F32, tag="rt", name="rt")
                            nc.vector.reciprocal_approx_fast(out=rt, in_=srow)
                            rb = work.tile([D_HEAD, 512], F32, tag="rb", name="rb")
                            nc.gpsimd.partition_broadcast(rb, rt)
                            avt = avsb.tile([D_HEAD, 512], BF16, tag="avt", name="avt")
                            nc.vector.tensor_mul(avt, avps[h][0:D_HEAD, :], rb)
                            # write into this batch's A2A buffer, split on
                            # token-chunk bounds
                            seg = 0
                            while seg < IB:
                                g = ib * IB + seg
                                chunk, coff = g // cs, g % cs
                                ln = min(IB - seg, cs - coff)
                                nc.sync.dma_start(
                                    out=av_in[b][chunk, h * D_HEAD : (h + 1) * D_HEAD, coff : coff + ln],
                                    in_=avt[:, seg : seg + ln],
                                )
                                seg += ln

                    # ---- AllToAll for this batch ----
                    if no_collective:
                        for k in range(n_cores):
                            nc.sync.dma_start(out=av_out[b][k], in_=av_in[b][k])
                    else:
                        nc.gpsimd.collective_compute(
                            "AllToAll",
                            mybir.AluOpType.bypass,
                            replica_groups=[list(range(n_cores))],
                            ins=[av_in[b].opt()],
                            outs=[av_out[b].opt()],
                        )

                    # ---- output projection + residual + LayerNorm for this
                    # batch's token rows (overlaps the next batch's attention)
                    if _rep == 0 and b == 0:
                        # late-phase constants (emitted here so the DMA queues
                        # serve hT and the qk/v weights first at kernel start)
                        for k in range(KCH):
                            nc.sync.dma_start(out=wo_sb[:, k, :], in_=wo_d[k])
                        for it in range(n_it):
                            nc.sync.dma_start(out=hres_sb[:, it, :], in_=hres_d[it])
                        if apply_gb:
                            nc.sync.dma_start(
                                out=g_sb,
                                in_=bass.AP(tensor=g_d.tensor, offset=g_d.offset, ap=[[0, P], *g_d.ap]),
                            )
                            nc.sync.dma_start(
                                out=b_sb,
                                in_=bass.AP(tensor=b_d.tensor, offset=b_d.offset, ap=[[0, P], *b_d.ap]),
                            )

                    avg_sb = qkvp.tile([P, n_cores, cs], BF16, tag="avg", bufs=2, name="avg_sb")
                    for k in range(n_cores):
                        nc.sync.dma_start(out=avg_sb[:, k, :], in_=av_out[b][k])

                    for i2 in range(tiles_pb):
                        it = b * tiles_pb + i2
                        # for the last batch (no attention left to overlap),
                        # alternate PSUM pools so tile it+1's matmuls pipeline
                        # with tile it's LayerNorm; earlier batches must leave
                        # the "av" slots to the next batch's attention
                        if b == B - 1:
                            popool = pproj if po_parity == 0 else pav
                            potag = "proj" if po_parity == 0 else "av"
                            po_parity ^= 1
                        else:
                            popool, potag = pproj, "proj"
                        pos = [
                            popool.tile([P, 512], F32, tag=potag, name=f"po{nh}")
                            for nh in range(2)
                        ]
                        for nh in range(2):
                            for k in range(n_cores):
                                nc.tensor.matmul(
                                    pos[nh],
                                    lhsT=avg_sb[:, k, i2 * P : (i2 + 1) * P],
                                    rhs=wo_sb[:, k, nh * 512 : (nh + 1) * 512],
                                    start=(k == 0),
                                    stop=(k == n_cores - 1),
                                )
                        x = defer_pool.tile([P, D_MODEL], F32, tag="x", name="x")
                        for nh in range(2):
                            nc.vector.tensor_add(
                                x[:, nh * 512 : (nh + 1) * 512],
                                pos[nh],
                                hres_sb[:, it, nh * 512 : (nh + 1) * 512],
                            )
                        stats = work.tile([P, nsub, 6], F32, tag="stats", name="stats")
                        for s in range(nsub):
                            nc.vector.bn_stats(stats[:, s, :], x[:, s * 512 : (s + 1) * 512])
                        mv = defer_pool.tile([P, 2], F32, tag="mv", name="mv")
                        nc.vector.bn_aggr(mv, stats)
                        # the sqrt + scale are deferred to the kernel tail so
                        # the sqrt ACT-table load doesn't thrash with the
                        # attention exps (different table sets)
                        deferred.append((it, x, mv))

                # ---- deferred LayerNorm tails (one sqrt table switch) ----
                for it, x, mv in deferred:
                    std = work.tile([P, 1], F32, tag="std", name="std")
                    nc.scalar.activation(
                        std, mv[:, 1:2], mybir.ActivationFunctionType.Sqrt, bias=eps_sb
                    )
                    rstd = work.tile([P, 1], F32, tag="rstd", name="rstd")
                    nc.vector.reciprocal(rstd, std)
                    xn = work.tile([P, D_MODEL], F32, tag="xn", name="xn")
                    nc.vector.tensor_scalar(
                        out=xn,
                        in0=x,
                        scalar1=mv[:, 0:1],
                        scalar2=rstd,
                        op0=mybir.AluOpType.subtract,
                        op1=mybir.AluOpType.mult,
                    )
                    if apply_gb:
                        nc.vector.tensor_mul(xn, xn, g_sb)
                        nc.vector.tensor_add(xn, xn, b_sb)
                    nc.sync.dma_start(out=out_d[it], in_=xn)
                deferred.clear()

    nc.compile()
    return nc


def make_in_maps(h, Wq, Wkv, Wo, ln_g, ln_b, t=T_FULL, n_cores=N_CORES):
    """Builds the per-core input maps (host-side sharding/layout prep)."""
    bf = ml_dtypes.bfloat16
    nh_loc = N_HEAD // n_cores
    cs = t // n_cores
    n_it = B * cs // P

    # hT: [B, KCH, P, t] = h transposed per batch, bf16 (shared by all cores)
    hT = np.ascontiguousarray(h.transpose(1, 2, 0)).reshape(B, KCH, P, t).astype(bf)
    # residual in batch-major token order
    h_bmaj = np.ascontiguousarray(h.transpose(1, 0, 2)).reshape(B * t, D_MODEL)
    g = np.ascontiguousarray(ln_g, dtype=np.float32)
    bvec = np.ascontiguousarray(ln_b, dtype=np.float32)
    wo = np.ascontiguousarray(Wo).reshape(KCH, P, D_MODEL).astype(bf)

    in_maps = []
    for c in range(n_cores):
        heads = [c * nh_loc + i for i in range(nh_loc)]
        # Wq columns for my heads, with the 1/sqrt(d) scale folded in
        wq_cols = [Wq[:, hd * D_HEAD : (hd + 1) * D_HEAD] * SCALE for hd in heads]
        # Wkv: head hd occupies cols [hd*128, hd*128+64) = K, [+64, +128) = V
        wk_cols = [Wkv[:, hd * 2 * D_HEAD : hd * 2 * D_HEAD + D_HEAD] for hd in heads]
        wv_cols = [Wkv[:, hd * 2 * D_HEAD + D_HEAD : (hd + 1) * 2 * D_HEAD] for hd in heads]
        wqk = np.concatenate(wq_cols + wk_cols, axis=1)  # [1024, 256]
        wv = np.concatenate(wv_cols, axis=1)  # [1024, 128]
        hres = np.concatenate(
            [h_bmaj[b * t + c * cs : b * t + (c + 1) * cs] for b in range(B)]
        ).reshape(n_it, P, D_MODEL)
        in_maps.append(
            {
                "hT": hT,
                "wqk": np.ascontiguousarray(wqk.reshape(KCH, P, 2 * nh_loc * D_HEAD)).astype(bf),
                "wv": np.ascontiguousarray(wv.reshape(KCH, P, nh_loc * D_HEAD)).astype(bf),
                "wo": wo,
                "hres": np.ascontiguousarray(hres, dtype=np.float32),
                "lng": g,
                "lnb": bvec,
            }
        )
    return in_maps


def assemble_output(results, t=T_FULL, n_cores=N_CORES):
    cs = t // n_cores
    chunks = [results[c]["out"].reshape(B, cs, D_MODEL) for c in range(n_cores)]
    # chunks[c][b] = batch-b tokens [c*cs, (c+1)*cs)
    full = np.concatenate(chunks, axis=1)  # [B, t, D]
    return np.ascontiguousarray(full.transpose(1, 0, 2))


def _numpy_fallback(h, attn_mask, Wq, Wkv, Wo, ln_g, ln_b):
    """Exact reference computation (only used if the mask is not causal)."""
    t, b, _ = h.shape
    hf = h.reshape(t * b, D_MODEL)
    q = (hf @ Wq).reshape(t, b, N_HEAD, D_HEAD)
    kv = (hf @ Wkv).reshape(t, b, N_HEAD, 2 * D_HEAD)
    k, v = kv[..., :D_HEAD], kv[..., D_HEAD:]
    s = np.einsum("ibnd,jbnd->ijbn", q, k) * SCALE
    s = np.where(attn_mask[:, :, :, None], -np.inf, s)
    s = s - s.max(axis=1, keepdims=True)
    p = np.exp(s)
    p = p / p.sum(axis=1, keepdims=True)
    av = np.einsum("ijbn,jbnd->ibnd", p, v).reshape(t, b, N_HEAD * D_HEAD)
    ao = av @ Wo
    x = h + ao
    mu = x.mean(axis=-1, keepdims=True)
    var = ((x - mu) ** 2).mean(axis=-1, keepdims=True)
    return ((x - mu) / np.sqrt(var + LN_EPS) * ln_g + ln_b).astype(np.float32)


_PROGRAM_CACHE = {}


def kernel(h, attn_mask, Wq, Wkv, Wo, ln_g, ln_b):
    global LAST_RESULT
    h = np.asarray(h, dtype=np.float32)
    attn_mask = np.asarray(attn_mask)
    Wq = np.asarray(Wq, dtype=np.float32)
    Wkv = np.asarray(Wkv, dtype=np.float32)
    Wo = np.asarray(Wo, dtype=np.float32)
    ln_g = np.asarray(ln_g, dtype=np.float32)
    ln_b = np.asarray(ln_b, dtype=np.float32)

    t = h.shape[0]
    causal = np.triu(np.ones((t, t), dtype=bool), k=1)
    if not np.array_equal(attn_mask, np.broadcast_to(causal[:, :, None], attn_mask.shape)):
        return _numpy_fallback(h, attn_mask, Wq, Wkv, Wo, ln_g, ln_b)

    apply_gb = not (np.all(ln_g == 1.0) and np.all(ln_b == 0.0))
    key = (t, apply_gb)
    if key not in _PROGRAM_CACHE:
        _PROGRAM_CACHE[key] = build_program(t=t, apply_gb=apply_gb)
    nc = _PROGRAM_CACHE[key]

    in_maps = make_in_maps(h, Wq, Wkv, Wo, ln_g, ln_b, t=t)
    res = run_bass_kernel_spmd(
        nc,
        in_maps,
        core_ids=list(range(N_CORES)),
        trace=bool(int(os.environ.get("KERNEL_TRACE", "0"))),
    )
    LAST_RESULT = res
    return assemble_output(res.results, t=t)


if __name__ == "__main__":
    # quick smoke: random small check vs numpy fallback path is not possible
    # (device required); just build the program.
    build_program()
    print("program built ok")



# revision 20
# speedup vs baseline: 3.9060x; 3.9060x over previous
"""Trainium2 Bass kernel for nn_MultiHeadAttn (dense transformer block:
QKV proj -> causal MHA -> out proj -> residual -> LayerNorm).

Sharding: tensor-parallel over the 16 heads across 8 NeuronCores (2 heads
per core). Each core computes Q/K/V for its heads over all tokens, causal
attention with the softmax denominator carried as an appended ones-column
in V, then an AllToAll redistributes the per-head attention vectors so
each core holds all 16 heads for 1/8 of the token rows and applies the
output projection, residual add and LayerNorm for those rows.

The body is structured as a small number of For_i hardware loops with all
varying-operand matmul inputs staged through fixed SBUF addresses (the PE
stationary operand cannot take register offsets), which keeps the static
instruction count low:
  - one QK projection loop over 512-token blocks (weights stationary),
  - one V projection loop over 128-token tiles producing V token-major,
  - four (batch, head) attention loops over the 16 key tiles, computing
    scores for the full query range and masking via a sliding causal-mask
    window, accumulating AV in PSUM across iterations,
  - one output-projection + LayerNorm loop over 128-row output tiles.
"""

import os
import sys

import numpy as np

try:
    import concourse.bass as bass  # noqa: F401
except ImportError:  # pragma: no cover
    sys.path.insert(0, "/opt/trn_rl_repo")

import ml_dtypes

import concourse.bass as bass
from concourse.bass import ds, ts
import concourse.mybir as mybir
import concourse.tile as tile
from concourse import bacc
from concourse.bass_utils import run_bass_kernel_spmd
from concourse.masks import make_upper_triangular

# Problem constants
T_FULL = 2048
B = 2
D_MODEL = 1024
N_HEAD = 16
D_HEAD = 64
LN_EPS = 1e-5
N_CORES = 8
SCALE = 1.0 / (D_HEAD**0.5)
EXP_BIAS = -3.0  # scores are in [-3.3, 3.3] for this problem; keeps exp <= ~1.4

P = 128
KCH = D_MODEL // P  # 8 contraction chunks

F32 = mybir.dt.float32
BF16 = mybir.dt.bfloat16

# Stash of the most recent run's BassKernelResults (for test harnesses).
LAST_RESULT = None


def build_program(t=T_FULL, n_cores=N_CORES, repeat=1, no_collective=False, apply_gb=True,
                  debug=False):
    """Builds the SPMD Bass program (same program on every core).

    repeat > 1 re-emits the whole kernel body (everything except constant
    weight loads) that many times — used only for wall-clock timing.
    """
    nh_loc = N_HEAD // n_cores  # 2 heads per core
    assert nh_loc == 2
    bt = B * t  # flattened (batch, token) axis, batch-major
    cs = t // n_cores  # per-batch token chunk per core (A2A shard)
    n_it = B * cs // P  # 128-row output tiles per core (4)
    njt = t // P  # key tiles per batch (16)
    nqb = t // 512  # query blocks per batch (4)

    nc = bacc.Bacc(
        "TRN2", target_bir_lowering=False, debug=False, num_devices=n_cores
    )

    # Kernel I/O (per-core tensors; host supplies per-core contents)
    hT_d = nc.dram_tensor("hT", [KCH, P, bt], BF16, kind="ExternalInput").ap()
    wq_d = nc.dram_tensor("wq", [KCH, P, P], BF16, kind="ExternalInput").ap()
    wk_d = nc.dram_tensor("wk", [KCH, P, P], BF16, kind="ExternalInput").ap()
    wv_d = nc.dram_tensor("wv", [KCH, P, P], BF16, kind="ExternalInput").ap()
    wo_d = nc.dram_tensor("wo", [KCH, P, D_MODEL], BF16, kind="ExternalInput").ap()
    hres_d = nc.dram_tensor("hres", [n_it, P, D_MODEL], F32, kind="ExternalInput").ap()
    g_d = nc.dram_tensor("lng", [D_MODEL], F32, kind="ExternalInput").ap()
    b_d = nc.dram_tensor("lnb", [D_MODEL], F32, kind="ExternalInput").ap()
    out_d = nc.dram_tensor("out", [n_it, P, D_MODEL], F32, kind="ExternalOutput").ap()
    if debug:
        qkT_dbg = nc.dram_tensor("qkT_dbg", [2, D_HEAD, N_HEAD // n_cores, bt], BF16,
                                 kind="ExternalOutput").ap()
        vext_dbg = nc.dram_tensor("vext_dbg", [P, B * njt, nh_loc, D_HEAD + 1], BF16,
                                  kind="ExternalOutput").ap()
        avn_dbg = nc.dram_tensor("avn_dbg", [B * nh_loc, D_HEAD, t], BF16,
                                 kind="ExternalOutput").ap()
        sum_dbg = nc.dram_tensor("sum_dbg", [B * nh_loc, 1, t], F32,
                                 kind="ExternalOutput").ap()

    with tile.TileContext(nc) as tc:
        with (
            tc.tile_pool(name="consts", bufs=1) as consts,
            tc.tile_pool(name="ps", bufs=1, space="PSUM") as psp,
            tc.tile_pool(name="dram", bufs=1, space="DRAM") as dram,
        ):
            # ---- one-time constants ----
            wq_sb = consts.tile([P, KCH, P], BF16)
            wk_sb = consts.tile([P, KCH, P], BF16)
            wv_sb = consts.tile([P, KCH, P], BF16)
            wo_sb = consts.tile([P, KCH, D_MODEL], BF16)
            nc.sync.dma_start(out=wq_sb, in_=wq_d.transpose([1, 0, 2]))
            nc.sync.dma_start(out=wk_sb, in_=wk_d.transpose([1, 0, 2]))
            nc.sync.dma_start(out=wv_sb, in_=wv_d.transpose([1, 0, 2]))
            nc.sync.dma_start(out=wo_sb, in_=wo_d.transpose([1, 0, 2]))
            hres_sb = consts.tile([P, n_it, D_MODEL], F32)
            nc.sync.dma_start(out=hres_sb, in_=hres_d.transpose([1, 0, 2]))
            if apply_gb:
                g_sb = consts.tile([P, D_MODEL], F32)
                b_sb = consts.tile([P, D_MODEL], F32)
                nc.sync.dma_start(
                    out=g_sb,
                    in_=bass.AP(tensor=g_d.tensor, offset=g_d.offset, ap=[[0, P], *g_d.ap]),
                )
                nc.sync.dma_start(
                    out=b_sb,
                    in_=bass.AP(tensor=b_d.tensor, offset=b_d.offset, ap=[[0, P], *b_d.ap]),
                )

            eps_sb = consts.tile([P, 1], F32)
            nc.vector.memset(eps_sb, LN_EPS)
            expb_sb = consts.tile([P, 1], F32)
            nc.vector.memset(expb_sb, EXP_BIAS)

            # sliding causal mask: W[j, c] = 1.0 iff c >= 2048 + j, so the
            # window W[:, 2048 - jt*128 :][:, :2048] keeps (jt*128 + j) <= q
            W_sb = consts.tile([P, 2 * t], BF16)
            nc.gpsimd.memset(W_sb[:, 0 : t - P], 0.0)
            make_upper_triangular(nc, W_sb[:, t - P : t], val=1.0, diag=True)
            nc.gpsimd.memset(W_sb[:, t : 2 * t], 1.0)
            # NOTE: W[:, t-P : t] has 1 where j <= c-(t-P), i.e. the diagonal
            # block; window start offset for key tile jt is t - (jt+1)*128.

            # ---- persistent work tiles (written each repeat) ----
            h_sb = consts.tile([P, KCH, bt], BF16)       # h^T, dmodel-major
            qT_sb = consts.tile([D_HEAD, nh_loc, bt], BF16)  # per-head, base 0
            kT_sb = consts.tile([D_HEAD, nh_loc, bt], BF16)
            vext = consts.tile([P, B * njt, nh_loc, D_HEAD + 1], BF16)
            nc.vector.memset(vext[:, :, :, D_HEAD : D_HEAD + 1], 1.0)
            hstage = consts.tile([P, KCH, P], BF16)
            astage = consts.tile([P, KCH, P], BF16)
            kstage = consts.tile([D_HEAD, P], BF16)
            vstage = consts.tile([P, D_HEAD + 1], BF16)
            expt = consts.tile([P, t], BF16)
            srow = consts.tile([1, t], F32)
            rt = consts.tile([1, t], F32)
            rb = consts.tile([D_HEAD, t], F32)
            avn = consts.tile([D_HEAD, t], BF16)
            x_sb = consts.tile([P, D_MODEL], F32)
            xn_sb = consts.tile([P, D_MODEL], F32)
            stats = consts.tile([P, 2, 6], F32)
            mv = consts.tile([P, 2], F32)
            std = consts.tile([P, 1], F32)
            rstd = consts.tile([P, 1], F32)

            psA = psp.tile([P, 4 * 512], F32, tag="A")  # 4 banks
            psB = psp.tile([P, 4 * 512], F32, tag="B")  # 4 banks

            for _rep in range(repeat):
                # ---- load h^T ----
                nc.sync.dma_start(out=h_sb, in_=hT_d.transpose([1, 0, 2]))

                # ---- QK projections: loop over 512-token blocks ----
                with tc.For_i(0, bt // 512) as i:
                    for mt, (wsb, dst) in enumerate(((wq_sb, qT_sb), (wk_sb, kT_sb))):
                        pslice = psA[:, mt * 512 : (mt + 1) * 512]
                        for k in range(KCH):
                            nc.tensor.matmul(
                                pslice,
                                lhsT=wsb[:, k, :],
                                rhs=h_sb[:, k, ts(i, 512)],
                                start=(k == 0),
                                stop=(k == KCH - 1),
                            )
                        for hl in range(nh_loc):
                            nc.vector.tensor_copy(
                                dst[:, hl, ts(i, 512)],
                                pslice[hl * D_HEAD : (hl + 1) * D_HEAD, :],
                            )

                # ---- V projection token-major: loop over 128-token tiles ----
                with tc.For_i(0, bt // P) as i:
                    nc.sync.dma_start(out=hstage, in_=h_sb[:, :, ts(i, P)])
                    vps = psA[:, 2 * 512 : 2 * 512 + P]
                    for k in range(KCH):
                        nc.tensor.matmul(
                            vps,
                            lhsT=hstage[:, k, :],
                            rhs=wv_sb[:, k, :],
                            start=(k == 0),
                            stop=(k == KCH - 1),
                        )
                    nc.vector.tensor_copy(
                        vext[:, ds(i, 1), :, 0:D_HEAD].squeeze(1), vps
                    )

                # ---- A2A buffers ----
                av_in = dram.tile([n_cores, P, B * cs], BF16,
                                  name=f"avin{_rep}")
                av_out = dram.tile([n_cores, P, B * cs], BF16,
                                   name=f"avout{_rep}")

                # ---- attention: per (batch, head), loop over key tiles ----
                for b in range(B):
                    for h in range(nh_loc):
                        pbase = h * D_HEAD
                        avps = psB[0 : D_HEAD + 1, :]
                        nc.vector.memset(avps, 0.0)
                        with tc.For_i(0, njt) as jt:
                            nc.sync.dma_start(
                                out=kstage,
                                in_=kT_sb[:, h, ds(jt * P + b * t, P)],
                            )
                            nc.sync.dma_start(
                                out=vstage,
                                in_=vext[:, ds(jt + b * njt, 1), h, :].squeeze(1),
                            )
                            for ib in range(nqb):
                                nc.tensor.matmul(
                                    psA[:, ib * 512 : (ib + 1) * 512],
                                    lhsT=kstage,
                                    rhs=qT_sb[:, h,
                                              b * t + ib * 512 : b * t + (ib + 1) * 512],
                                    start=True,
                                    stop=True,
                                )
                            nc.scalar.activation(
                                expt, psA, mybir.ActivationFunctionType.Exp,
                                bias=expb_sb,
                            )
                            nc.vector.tensor_mul(
                                expt, expt, W_sb[:, ds(jt * (-P) + t - P, t)]
                            )
                            for ib in range(nqb):
                                nc.tensor.matmul(
                                    avps[:, ib * 512 : (ib + 1) * 512],
                                    lhsT=vstage,
                                    rhs=expt[:, ib * 512 : (ib + 1) * 512],
                                    start=False,
                                    stop=True,
                                    skip_group_check=True,
                                )
                        # normalize by sumexp (psum row 64) and ship to the
                        # A2A buffer: av_in[chunk, h*64+d, b*cs + t']
                        nc.vector.tensor_copy(srow, avps[D_HEAD : D_HEAD + 1, :])
                        nc.vector.reciprocal_approx_fast(out=rt, in_=srow)
                        nc.gpsimd.partition_broadcast(rb, rt)
                        nc.vector.tensor_mul(avn, avps[0:D_HEAD, :], rb)
                        nc.sync.dma_start(
                            out=av_in[:, pbase : pbase + D_HEAD, b * cs : (b + 1) * cs]
                            .transpose([1, 0, 2]),
                            in_=bass.AP(
                                tensor=avn.tensor,
                                offset=avn.offset,
                                ap=[[avn.ap[0][0], D_HEAD], [cs, n_cores], [1, cs]],
                            ),
                        )
                        if debug and _rep == 0:
                            nc.sync.dma_start(out=avn_dbg[b * nh_loc + h], in_=avn)
                            nc.sync.dma_start(out=sum_dbg[b * nh_loc + h], in_=srow)

                # ---- AllToAll ----
                if no_collective:
                    nc.sync.dma_start(out=av_out, in_=av_in)
                else:
                    nc.gpsimd.collective_compute(
                        "AllToAll",
                        mybir.AluOpType.bypass,
                        replica_groups=[list(range(n_cores))],
                        ins=[av_in.opt()],
                        outs=[av_out.opt()],
                    )

                if debug and _rep == 0:
                    nc.sync.dma_start(out=qkT_dbg[0], in_=qT_sb)
                    nc.sync.dma_start(out=qkT_dbg[1], in_=kT_sb)
                    nc.sync.dma_start(out=vext_dbg, in_=vext)

                # ---- output projection + residual + LayerNorm ----
                with tc.For_i(0, n_it) as i:
                    nc.sync.dma_start(
                        out=astage,
                        in_=av_out.transpose([1, 0, 2])[:, :, ds(i * P, P)],
                    )
                    wops = psA[:, 0:1024]
                    for half in range(2):
                        for k in range(KCH):
                            nc.tensor.matmul(
                                wops[:, half * 512 : (half + 1) * 512],
                                lhsT=astage[:, k, :],
                                rhs=wo_sb[:, k, half * 512 : (half + 1) * 512],
                                start=(k == 0),
                                stop=(k == KCH - 1),
                            )
                    nc.vector.tensor_add(
                        x_sb, wops, hres_sb[:, ds(i, 1), :].squeeze(1)
                    )
                    for s in range(2):
                        nc.vector.bn_stats(stats[:, s, :], x_sb[:, s * 512 : (s + 1) * 512])
                    nc.vector.bn_aggr(mv, stats)
                    nc.scalar.activation(
                        std, mv[:, 1:2], mybir.ActivationFunctionType.Sqrt,
                        bias=eps_sb,
                    )
                    nc.vector.reciprocal(rstd, std)
                    nc.vector.tensor_scalar(
                        out=xn_sb,
                        in0=x_sb,
                        scalar1=mv[:, 0:1],
                        scalar2=rstd,
                        op0=mybir.AluOpType.subtract,
                        op1=mybir.AluOpType.mult,
                    )
                    if apply_gb:
                        nc.vector.tensor_mul(xn_sb, xn_sb, g_sb)
                        nc.vector.tensor_add(xn_sb, xn_sb, b_sb)
                    nc.sync.dma_start(out=out_d[ds(i, 1)].squeeze(0), in_=xn_sb)

    nc.compile()
    return nc


def make_in_maps(h, Wq, Wkv, Wo, ln_g, ln_b, t=T_FULL, n_cores=N_CORES):
    """Builds the per-core input maps (host-side sharding/layout prep)."""
    bfd = ml_dtypes.bfloat16
    nh_loc = N_HEAD // n_cores
    cs = t // n_cores
    n_it = B * cs // P

    # hT: [KCH, P, B*t] = h as [dmodel, batch-major tokens], bf16 (shared)
    hT = np.ascontiguousarray(
        h.transpose(2, 1, 0).reshape(KCH, P, B * t)
    ).astype(bfd)
    wo = np.ascontiguousarray(Wo).reshape(KCH, P, D_MODEL).astype(bfd)
    g = np.ascontiguousarray(ln_g, dtype=np.float32)
    bvec = np.ascontiguousarray(ln_b, dtype=np.float32)

    in_maps = []
    for c in range(n_cores):
        heads = [c * nh_loc + i for i in range(nh_loc)]
        wq_cols = np.concatenate(
            [Wq[:, hd * D_HEAD : (hd + 1) * D_HEAD] * SCALE for hd in heads], axis=1
        )
        wk_cols = np.concatenate(
            [Wkv[:, hd * 2 * D_HEAD : hd * 2 * D_HEAD + D_HEAD] for hd in heads],
            axis=1,
        )
        wv_cols = np.concatenate(
            [Wkv[:, hd * 2 * D_HEAD + D_HEAD : (hd + 1) * 2 * D_HEAD] for hd in heads],
            axis=1,
        )
        # residual rows for my token chunks, batch-major: it = b*2 + i2
        hres = np.concatenate(
            [
                h[b * 0 + c * cs : c * cs + cs, b, :].reshape(cs // P, P, D_MODEL)
                for b in range(B)
            ]
        ).reshape(n_it, P, D_MODEL)
        in_maps.append(
            {
                "hT": hT,
                "wq": np.ascontiguousarray(wq_cols.reshape(KCH, P, P)).astype(bfd),
                "wk": np.ascontiguousarray(wk_cols.reshape(KCH, P, P)).astype(bfd),
                "wv": np.ascontiguousarray(wv_cols.reshape(KCH, P, P)).astype(bfd),
                "wo": wo,
                "hres": np.ascontiguousarray(hres, dtype=np.float32),
                "lng": g,
                "lnb": bvec,
            }
        )
    return in_maps


def assemble_output(results, t=T_FULL, n_cores=N_CORES):
    cs = t // n_cores
    full = np.empty((t, B, D_MODEL), dtype=np.float32)
    for c in range(n_cores):
        o = results[c]["out"].reshape(B, cs, D_MODEL)
        for b in range(B):
            full[c * cs : (c + 1) * cs, b, :] = o[b]
    return full


def _numpy_fallback(h, attn_mask, Wq, Wkv, Wo, ln_g, ln_b):
    """Exact reference computation (only used if the mask is not causal)."""
    t, b, _ = h.shape
    hf = h.reshape(t * b, D_MODEL)
    q = (hf @ Wq).reshape(t, b, N_HEAD, D_HEAD)
    kv = (hf @ Wkv).reshape(t, b, N_HEAD, 2 * D_HEAD)
    k, v = kv[..., :D_HEAD], kv[..., D_HEAD:]
    s = np.einsum("ibnd,jbnd->ijbn", q, k) * SCALE
    s = np.where(attn_mask[:, :, :, None], -np.inf, s)
    s = s - s.max(axis=1, keepdims=True)
    p = np.exp(s)
    p = p / p.sum(axis=1, keepdims=True)
    av = np.einsum("ijbn,jbnd->ibnd", p, v).reshape(t, b, N_HEAD * D_HEAD)
    ao = av @ Wo
    x = h + ao
    mu = x.mean(axis=-1, keepdims=True)
    var = ((x - mu) ** 2).mean(axis=-1, keepdims=True)
    return ((x - mu) / np.sqrt(var + LN_EPS) * ln_g + ln_b).astype(np.float32)


_PROGRAM_CACHE = {}


def kernel(h, attn_mask, Wq, Wkv, Wo, ln_g, ln_b):
    global LAST_RESULT
    h = np.asarray(h, dtype=np.float32)
    attn_mask = np.asarray(attn_mask)
    Wq = np.asarray(Wq, dtype=np.float32)
    Wkv = np.asarray(Wkv, dtype=np.float32)
    Wo = np.asarray(Wo, dtype=np.float32)
    ln_g = np.asarray(ln_g, dtype=np.float32)
    ln_b = np.asarray(ln_b, dtype=np.float32)

    t = h.shape[0]
    causal = np.triu(np.ones((t, t), dtype=bool), k=1)
    if not np.array_equal(attn_mask, np.broadcast_to(causal[:, :, None], attn_mask.shape)):
        return _numpy_fallback(h, attn_mask, Wq, Wkv, Wo, ln_g, ln_b)

    apply_gb = not (np.all(ln_g == 1.0) and np.all(ln_b == 0.0))
    key = (t, apply_gb)
    if key not in _PROGRAM_CACHE:
        _PROGRAM_CACHE[key] = build_program(t=t, apply_gb=apply_gb)
    nc = _PROGRAM_CACHE[key]

    in_maps = make_in_maps(h, Wq, Wkv, Wo, ln_g, ln_b, t=t)
    res = run_bass_kernel_spmd(
        nc,
        in_maps,
        core_ids=list(range(N_CORES)),
        trace=bool(int(os.environ.get("KERNEL_TRACE", "0"))),
    )
    LAST_RESULT = res
    return assemble_output(res.results, t=t)


if __name__ == "__main__":
    build_program()
    print("program built ok")
